# revision 2
# baseline (speedup 1.0000x reference)
"""Correlation kernel for Trainium2 (Bass/Tile), 8 NeuronCores.

Problem: inputs (B=4, N=2, C=128, H=128, W=128) fp32.
  src = inputs[:, 0], target = inputs[:, 1]
  out[b, k, y, x] = (1/C) * sum_c src[b,c,y,x] * target[b,c,y+dy,x+dx]
  for k = (dy+10)*21 + (dx+10), dy,dx in [-10,10], zero-padded target.
  Output (4, 441, 128, 128) fp32.

Mapping (v2 — single bf16 pass + on-chip diagonal extraction):
  - Shard over 8 cores: (b in 0..3) x (H half in 0..1). Each core handles
    64 output rows; halos come from host-side padded slabs.
  - Per output row y, contraction over C runs on the PE:
      stationary = src row chunk (128c x 32x), col-tiled at tile_position
      (0, 32g) so 4 x-chunks share the array;
      moving = padded target rows [y+dy', x-window 52 wide] with the AP
      rearranged to (u, dy') order, 7 dy per matmul (N = 52*7 = 364).
    PSUM tile per y: [128, 3, 512] (3 banks); free layout (u, dy) so the
    21 dy values for one u are contiguous after re-assembly.
  - DVE/ScalarE evacuate PSUM -> SBUF window tile [128, YC, 52, 21] in
    bf16 (fp32->bf16 cast in the copy). In the flattened (u*21+dy) free
    index, the 441 values needed by partition x = 32g+xi are exactly the
    contiguous span [xi*21, xi*21+441).
  - Output DMA: 32 "diagonal" DMAs per chunk (one per xi in 0..31), each
    reading partitions {xi, 32+xi, 64+xi, 96+xi} (partition stride 32 via
    rearrange) at free offset xi*21, with contiguous 441*2B runs, into a
    DRAM tensor [128x, 64y, 441] bf16. Total out traffic 7.2 MB/core vs
    35.8 MB for the fp32 window scheme.
  - Host re-indexes (x, y, dx, dy) -> (dy*21+dx, y, x) and casts fp32.
  Numerics: single bf16 pass (inputs rounded to bf16); rel l2 err ~4e-3,
  within the 2e-2 gate. Scaling by 1/C folded into src on host (exact).
"""

import os

import ml_dtypes
import numpy as np

import concourse.bacc as bacc
import concourse.bass as bass
import concourse.mybir as mybir
import concourse.tile as tile
from concourse.bass_utils import run_bass_kernel_spmd

B = 4
C = 128
H = 128
W = 128
KS = 21          # kernel size (per axis)
P = KS // 2      # pad / max displacement = 10
HY = H // 2      # rows per core = 64
NG = 4           # x groups (col-tiling), 32 wide each
GW = 32          # group width
WIN = GW + 2 * P     # 52: target x-window per group
DYB = 3          # dy batches
DYI = KS // DYB  # 7 dy per batch
NMOV = DYI * WIN     # 364 moving columns per matmul
TGT_H = HY + 2 * P   # 84 target rows per core
TGT_W = W + 2 * P    # 148 padded target width
OUTF = WIN * KS      # 1092 window values per (y, x)
YC = 16              # output rows per window chunk / store batch

_CACHE = {}


def _build_module():
    """Build the SPMD Bass module (same program on all 8 cores)."""
    f32 = mybir.dt.float32
    bf16 = mybir.dt.bfloat16
    nc = bacc.Bacc("TRN2", target_bir_lowering=False, debug=False)

    src_d = nc.declare_dram_parameter("src", [C, HY, W], bf16, isOutput=False)
    tgt_d = nc.declare_dram_parameter("tgt", [C, TGT_H, TGT_W], bf16,
                                      isOutput=False)
    out_d = nc.declare_dram_parameter("out_ext", [128, HY, KS * KS], bf16,
                                      isOutput=True)

    with tile.TileContext(nc) as tc:
        with (
            tc.tile_pool(name="inp", bufs=1) as inp,
            tc.tile_pool(name="psum", bufs=2, space=bass.MemorySpace.PSUM) as psum,
            tc.tile_pool(name="win", bufs=2) as winp,
        ):
            src_sb = inp.tile([C, HY, W], bf16, name="sb_src")
            tgt_sb = inp.tile([C, TGT_H, TGT_W], bf16, name="sb_tgt")
            # Split loads so early rows' matmuls can start before the whole
            # slab lands.
            nchunk = 8
            for i in range(nchunk):
                ys = (TGT_H + nchunk - 1) // nchunk
                lo = i * ys
                hi = min(TGT_H, lo + ys)
                nc.sync.dma_start(tgt_sb[:, lo:hi, :], tgt_d[:, lo:hi, :])
                ys = (HY + nchunk - 1) // nchunk
                lo = i * ys
                hi = min(HY, lo + ys)
                nc.sync.dma_start(src_sb[:, lo:hi, :], src_d[:, lo:hi, :])

            out_r = out_d.rearrange("(g xi) y f -> xi g y f", xi=GW)

            for yc in range(HY // YC):
                win = winp.tile([128, YC, WIN, KS], bf16)
                for yy in range(YC):
                    y = yc * YC + yy
                    ps = psum.tile([128, DYB, 512], f32)
                    for dyb in range(DYB):
                        for g in range(NG):
                            lhsT = src_sb[:, y, g * GW:(g + 1) * GW]
                            rhs = tgt_sb[:, y + dyb * DYI:
                                         y + (dyb + 1) * DYI,
                                         g * GW: g * GW + WIN]
                            nc.tensor.matmul(
                                ps[g * GW:(g + 1) * GW, dyb, 0:NMOV],
                                lhsT,
                                rhs.rearrange("c r u -> c u r"),
                                start=True,
                                stop=True,
                                tile_position=(0, g * GW),
                            )
                    # Evacuate + cast + transpose to (u, dy) layout; the
                    # three dy-batches interleave into the dy axis.
                    for dyb in range(DYB):
                        src_ap = ps[:, dyb, 0:NMOV].rearrange(
                            "p (u d) -> p u d", d=DYI)
                        dst_ap = win[:, yy, :, dyb * DYI:(dyb + 1) * DYI]
                        if (y * DYB + dyb) % 2 == 0:
                            nc.vector.tensor_copy(dst_ap, src_ap)
                        else:
                            nc.scalar.copy(dst_ap, src_ap)
                # Diagonal extraction: one DMA per xi; partitions xi::32,
                # contiguous 441-value run at free offset xi*21.
                win_r = win[:].rearrange("(g xi) y u d -> xi g y (u d)",
                                         xi=GW)
                for xi in range(GW):
                    nc.sync.dma_start(
                        out_r[xi, :, yc * YC:(yc + 1) * YC, :],
                        win_r[xi, :, :, xi * KS: xi * KS + KS * KS],
                    )

    nc.compile()
    return nc


def _get_module():
    if "v2" not in _CACHE:
        _CACHE["v2"] = _build_module()
    return _CACHE["v2"]


def _shard_inputs(inputs: np.ndarray):
    src = inputs[:, 0] * np.float32(1.0 / C)  # exact power of two
    tgt = inputs[:, 1]
    src_bf = src.astype(ml_dtypes.bfloat16)
    tgt_pad = np.pad(tgt, ((0, 0), (0, 0), (P, P), (P, P))).astype(
        ml_dtypes.bfloat16)
    in_maps = []
    for core in range(8):
        b, h = divmod(core, 2)
        m = {
            "src": np.ascontiguousarray(src_bf[b, :, h * HY:(h + 1) * HY, :]),
            "tgt": np.ascontiguousarray(
                tgt_pad[b, :, h * HY: h * HY + TGT_H, :]),
        }
        in_maps.append(m)
    return in_maps


def run(inputs: np.ndarray, trace: bool = False, mode: str | None = None):
    nc = _get_module()
    in_maps = _shard_inputs(inputs)
    res = run_bass_kernel_spmd(
        nc, in_maps, core_ids=list(range(8)), trace=trace,
    )
    out = np.empty((B, KS * KS, H, W), dtype=np.float32)
    for core in range(8):
        b, h = divmod(core, 2)
        r = np.asarray(res.results[core]["out_ext"]).astype(np.float32)
        # r: [x, y, dx*21+dy] -> out[k=dy*21+dx, y, x]
        r4 = r.reshape(128, HY, KS, KS)          # [x, y, dx, dy]
        blk = r4.transpose(3, 2, 1, 0).reshape(KS * KS, HY, 128)
        out[b, :, h * HY:(h + 1) * HY, :] = blk
    return out, res.exec_time_ns


def kernel(inputs: np.ndarray) -> np.ndarray:
    out, _ = run(np.asarray(inputs))
    return out
